# revision 28
# baseline (speedup 1.0000x reference)
"""Behler symmetry functions (set-51: 8 G2 + 43 G4) on 8 Trainium2 cores.

Sharding: data-parallel over atoms. Each core handles 250 atoms (2 tiles of
<=128 atoms on partitions); both tiles ride the free dim of most ops.

Host-side prep is pure data marshalling (no FLOPs): the neighbor positions
pos[neighs] are expanded into a contiguous per-core [P, 2, K, 3] block
(invalid slots -> far-away sentinel), so the device ingests one strided DMA
instead of 64 serial software-DGE indirect gathers (~1.04us each on the Pool
engine, which would dominate the kernel).  All arithmetic runs on-device.

Structure per core:
  - neighbor stage builds a 7-row stack [fc, invr, x, y, z, rsq, one]
    (j-side) and a k-side variant [.., one, rsq] so a single tensor_tensor
    per diagonal d produces fcprod/invprod/xx/yy/zz/rsqj/rsqk for the pairs
    (j, j+d); 31 diagonals cover the 496 unordered pairs.
  - G4 algebra:  s' = rsqj+rsqk-dot,  rjk^2 = s'-dot,  cos = dot*invprod,
    u = (1+cos)/2,  2^(1-z)*(1+lam*cos)^z = 2*((1+lam*cos)/2)^z, the factor
    2 folded into w = 2*fcij*fcik*fcjk = fcprod*(1-sin_term).
  - all 43 G4 outputs are linear combos of "moment cells"
    M[k,e] = sum_pairs (w*u^k) * exp(-2*eta_e*s'), k=0..4 (+ one u^16 cell).
    Cell products run as bf16 tensor_tensor (DVE 2x mode, some on Pool);
    the pair-reduction is split between a DVE bf16 halving tree (batched
    per k-row) and ACT activation accum_out (Identity with accumulate).
  - lam=-1 columns are tiny binomial combos of the moments.
"""

import sys

sys.path.insert(0, "/opt/trn_rl_repo")

import numpy as np

import concourse.bass as bass
import concourse.mybir as mybir
from concourse.bass import AP
from concourse.tile import TileContext
from concourse.bass_utils import run_bass_kernel_spmd

AF = mybir.ActivationFunctionType
ALU = mybir.AluOpType
DT = mybir.dt

N_ATOMS = 2000
K = 32
N_CORES = 8
APC = N_ATOMS // N_CORES          # atoms per core (250)
TILES = 2                         # partition tiles per core (128 + 122)
P = 128
RCUT = 8.0
NPAIR = K * (K - 1) // 2          # 496
FARPOS = 1.0e4                    # sentinel position (far away -> fc = 0)

G2_ETA = [0.0036, 0.036, 0.071, 0.125, 0.214, 0.357, 0.714, 1.428]
ETAS7 = [0.0001, 0.003, 0.008, 0.015, 0.025, 0.045, 0.08]

DIAG_OFF = []
_off = 0
for _d in range(1, K):
    DIAG_OFF.append(_off)
    _off += K - _d

# ---------------- tuning knobs ----------------------------------------------
# cell classes: power of u=(1+cos)/2 or v=(1-cos)/2 weighted by w, one cell
# per (eta, class); every cell accumulates straight into its OUT51 column.
#   class -> OUT51 column offset within the 6-column eta block
XCLASSES = ["v1", "u1", "v2", "u2", "v4", "u4"]   # offsets 0..5
# per class: etas 0..TREE_N[c]-1 reduce via the DVE bf16 tree,
# etas TREE_N[c]..6 via ACT Identity-accum.  The u16 cell always goes ACT.
TREE_N = {"v1": 5, "u1": 5, "v2": 5, "u2": 5, "v4": 4, "u4": 4}
# cell products that run on Pool (the rest run on DVE)
POOL_PRODUCTS = (
    {(0, "u1"), (0, "v1"), (1, "u1"), (1, "v1"), (2, "u1"), (2, "v1")}
    | {(5, "v1"), (6, "v1"), (5, "u1"), (6, "u1")}
    | {(5, "v2"), (6, "v2"), (5, "u2"), (6, "u2")}
    | {(3, "v4"), (4, "v4"), (3, "u4"), (4, "u4"), (5, "v4"), (5, "u4"),
       (6, "v4"), (6, "u4")}
)


def set_knobs(tree_n=None, pool_products=None, diag_pool_every=None):
    """Override tuning knobs and clear the cached module."""
    global TREE_N, POOL_PRODUCTS, DIAG_POOL_EVERY, _NC_CACHE
    if tree_n is not None:
        TREE_N = dict(tree_n)
    if pool_products is not None:
        POOL_PRODUCTS = set(pool_products)
    if diag_pool_every is not None:
        DIAG_POOL_EVERY = diag_pool_every
    _NC_CACHE = None
# of the 31 diagonals (longest first), every DIAG_POOL_EVERYth -> Pool
DIAG_POOL_EVERY = 3

# constant-table column values (bias APs for ACT + the negated G2 etas)
CTAB_VALS = [float(np.pi / 2), float(-np.pi / 2), 0.5, 0.0, 2e-4]
MAX_WAITS_PER_INST = 1


def _split_excess_waits(nc):
    """This toolchain rejects instructions carrying more than ~2 sem waits.
    Move excess waits onto NoOp carriers spliced before, same engine."""
    for fn in nc.m.functions:
        for bb in fn.blocks:
            new_list = []
            changed = False
            for inst in bb.instructions:
                si = inst.sync_info
                if si is not None and len(si.on_wait) > MAX_WAITS_PER_INST:
                    waits = list(si.on_wait)
                    extra = waits[:-MAX_WAITS_PER_INST]
                    keep = waits[-MAX_WAITS_PER_INST:]
                    for i in range(0, len(extra), MAX_WAITS_PER_INST):
                        nop = mybir.InstNoOp(
                            name=f"WS-{nc.next_id()}",
                            engine=inst.engine,
                            sync_info=mybir.SyncInfo(
                                on_wait=extra[i : i + MAX_WAITS_PER_INST], on_update=[]
                            ),
                            bass_nofuse=True,
                        )
                        nc.register_instruction(nop, overwrite=True)
                        new_list.append(nop)
                    inst.sync_info = mybir.SyncInfo(
                        on_wait=keep, on_update=list(si.on_update)
                    )
                    changed = True
                new_list.append(inst)
            if changed:
                bb.instructions = new_list


def _view(offset_ap, dims):
    """AP view anchored at an indexed element: dims = [[step, count], ...]."""
    return AP(offset_ap.tensor, offset_ap.offset, [offset_ap.ap[0]] + dims)


def build_nc():
    nc = bass.Bass()

    ct_in = nc.declare_dram_parameter("ctab", [P, 16], DT.float32, isOutput=False)
    gp_in = nc.declare_dram_parameter("gpos", [P, TILES, K, 3], DT.float32, isOutput=False)
    own_in = nc.declare_dram_parameter("own", [P, TILES, 3], DT.float32, isOutput=False)
    out_d = nc.declare_dram_parameter("out", [P, TILES, 51], DT.float32, isOutput=True)

    with TileContext(nc) as tc:
        with tc.tile_pool(name="main", bufs=1) as mp:
            _body(nc, tc, mp, gp_in, own_in, out_d, ct_in)

    _split_excess_waits(nc)
    return nc


def _body(nc, tc, mp, gp_in, own_in, out_d, ct_in):
    f32 = DT.float32
    bf16 = DT.bfloat16

    # G first: it gates the whole kernel, so it must win the HWDGE;
    # constants and own positions follow on the scalar queue.
    G = mp.tile([P, TILES, K, 3], f32)
    nc.sync.dma_start(out=G[:], in_=gp_in[:])
    ctab = mp.tile([P, 16], f32)
    nc.scalar.dma_start(out=ctab[:], in_=ct_in[:])
    for i, v in enumerate(CTAB_VALS):
        nc.const_aps.aps[(f32, v)] = ctab[:, i : i + 1]
    own_t = mp.tile([P, TILES, 3], f32)
    nc.scalar.dma_start(out=own_t[:], in_=own_in[:])

    # ---------------- neighbor stage ([P, 2, *, K]) -----------------------
    # j-side stack: rows 0 fc, 1 invr, 2 x, 3 y, 4 z, 5 rsq, 6 one
    Sm = mp.tile([P, TILES, 7, K], f32)
    # k-side stack: rows 0-4 same, 5 one, 6 rsq
    SmB = mp.tile([P, TILES, 7, K], f32)
    nc.gpsimd.memset(Sm[:, :, 6], 1.0)
    nc.gpsimd.memset(SmB[:, :, 5], 1.0)

    # rvec = G - own, written transposed straight into Sm rows 2..4
    own_b = _view(own_t[:, 0, 0], [[3, TILES], [0, K], [1, 3]])
    sm_xyz_t = _view(Sm[:, 0, 2, 0], [[7 * K, TILES], [1, K], [K, 3]])
    nc.vector.tensor_tensor(out=sm_xyz_t, in0=G[:], in1=own_b, op=ALU.subtract)

    SQ = mp.tile([P, TILES, 3, K], f32)
    nc.scalar.activation(SQ[:], Sm[:, :, 2:5], AF.Square)
    sq_kc = _view(SQ[:, 0, 0, 0], [[3 * K, TILES], [1, K], [K, 3]])
    smb_rsq = _view(SmB[:, 0, 6, 0], [[7 * K, TILES], [1, K]])
    nc.vector.tensor_reduce(out=smb_rsq, in_=sq_kc, axis=mybir.AxisListType.X, op=ALU.add)
    nc.gpsimd.tensor_copy(out=Sm[:, :, 5], in_=SmB[:, :, 6])

    r = mp.tile([P, TILES, K], f32)
    nc.scalar.activation(r[:], SmB[:, :, 6], AF.Sqrt)
    nc.vector.reciprocal(Sm[:, :, 1], r[:])
    rm = mp.tile([P, TILES, K], f32)
    nc.vector.tensor_scalar_min(rm[:], r[:], RCUT)
    sn = mp.tile([P, TILES, K], f32)
    nc.scalar.activation(
        sn[:], rm[:], AF.Sin, bias=float(-np.pi / 2), scale=float(np.pi / RCUT)
    )
    nc.vector.tensor_scalar(Sm[:, :, 0], sn[:], -0.5, 0.5, ALU.mult, ALU.add)
    nc.gpsimd.tensor_copy(out=SmB[:, :, 0:5], in_=Sm[:, :, 0:5])

    OUT51 = mp.tile([P, TILES, 51], f32)

    # ---------------- G2: broadcast exp + mult + segmented reduce ---------
    # ---------------- pair stage ------------------------------------------
    # G2 exp early (only needs rsq): Pool + ACT fill idle during the diag
    E2X = mp.tile([P, TILES, 8, K], f32)
    rsq_b = _view(Sm[:, 0, 5, 0], [[7 * K, TILES], [0, 8], [1, K]])
    eta_b = _view(ctab[:, 8], [[0, TILES], [1, 8], [0, K]])
    nc.gpsimd.tensor_tensor(out=E2X[:], in0=rsq_b, in1=eta_b, op=ALU.mult)
    E2 = mp.tile([P, TILES, 8, K], f32)
    nc.scalar.activation(E2[:], E2X[:], AF.Exp)

    # pair-major-inner layout [P, T, pair, row]: each diagonal's write window
    # is a contiguous disjoint byte range, so DVE- and Pool-issued diagonals
    # never alias in the hazard tracker and run fully concurrent.
    Mst = mp.tile([P, TILES, NPAIR, 7], f32)
    # chunk-A diagonals (d<=10) all on DVE so chunk A's chain starts early;
    # chunk-B diagonals alternate Pool/DVE
    for i, d in enumerate(range(1, K)):
        L = K - d
        o = DIAG_OFF[d - 1]
        in0 = _view(Sm[:, 0, 0, 0], [[7 * K, TILES], [K, 7], [1, L]])
        in1 = _view(SmB[:, 0, 0, d], [[7 * K, TILES], [K, 7], [1, L]])
        outp = _view(Mst[:, 0, o, 0], [[7 * NPAIR, TILES], [1, 7], [7, L]])
        if d <= 10:
            eng = nc.vector
        else:
            eng = nc.gpsimd if (i % 2) == 0 else nc.vector
        eng.tensor_tensor(out=outp, in0=in0, in1=in1, op=ALU.mult)

    def mrow(rr):
        return _view(Mst[:, 0, 0, rr], [[7 * NPAIR, TILES], [7, NPAIR]])

    def mrowr(rr, a, n):
        return _view(Mst[:, 0, a, rr], [[7 * NPAIR, TILES], [7, n]])

    PF = [TILES, NPAIR]
    # two pair-range chunks: A = diagonals 1..10, B = 11..31.  Chunk A's
    # chain/E/products overlap chunk B's diagonals and chain.
    CHUNKS = [(0, DIAG_OFF[10]), (DIAG_OFF[10], NPAIR - DIAG_OFF[10])]

    def rv(tile, a, n):
        return _view(tile[:, 0, a], [[NPAIR, TILES], [1, n]])

    dot = mp.tile([P] + PF, f32)
    sumr = mp.tile([P] + PF, f32)
    sp = mp.tile([P] + PF, f32)
    rjk2 = mp.tile([P] + PF, f32)
    rjk = mp.tile([P] + PF, f32)
    tmp = rjk  # scratch alias: tmp dies before rjk is written
    rm2 = mp.tile([P] + PF, f32)
    sn2 = mp.tile([P] + PF, f32)
    cos = mp.tile([P] + PF, f32)
    t2 = mp.tile([P] + PF, f32)
    ub = mp.tile([P] + PF, bf16)
    vb = mp.tile([P] + PF, bf16)
    wb = mp.tile([P] + PF, bf16)
    E = mp.tile([P, TILES, 7, NPAIR], bf16)

    def erv(e, a, n):
        return _view(E[:, 0, e, a], [[7 * NPAIR, TILES], [1, n]])

    for ci, (a, n) in enumerate(CHUNKS):
        ve = nc.vector
        ve.tensor_tensor(out=rv(tmp, a, n), in0=mrowr(2, a, n), in1=mrowr(3, a, n), op=ALU.add)
        ve.tensor_tensor(out=rv(dot, a, n), in0=rv(tmp, a, n), in1=mrowr(4, a, n), op=ALU.add)
        nc.gpsimd.tensor_tensor(out=rv(sumr, a, n), in0=mrowr(5, a, n), in1=mrowr(6, a, n), op=ALU.add)
        ve.tensor_tensor(out=rv(sp, a, n), in0=rv(sumr, a, n), in1=rv(dot, a, n), op=ALU.subtract)
        ve.tensor_tensor(out=rv(rjk2, a, n), in0=rv(sp, a, n), in1=rv(dot, a, n), op=ALU.subtract)
        nc.scalar.activation(rv(rjk, a, n), rv(rjk2, a, n), AF.Sqrt, bias=2e-4)
        ve.tensor_scalar_min(rv(rm2, a, n), rv(rjk, a, n), RCUT)
        nc.scalar.activation(
            rv(sn2, a, n), rv(rm2, a, n), AF.Sin,
            bias=float(-np.pi / 2), scale=float(np.pi / RCUT),
        )
        nc.gpsimd.tensor_tensor(out=rv(cos, a, n), in0=rv(dot, a, n), in1=mrowr(1, a, n), op=ALU.mult)
        ve.tensor_tensor(out=rv(t2, a, n), in0=mrowr(0, a, n), in1=rv(sn2, a, n), op=ALU.mult)
        nc.vector.scalar_tensor_tensor(
            out=rv(wb, a, n), in0=rv(t2, a, n), scalar=-1.0, in1=mrowr(0, a, n),
            op0=ALU.mult, op1=ALU.add,
        )
    nc.scalar.activation(ub[:], cos[:], AF.Relu, bias=0.5, scale=0.5)
    nc.scalar.activation(vb[:], cos[:], AF.Relu, bias=0.5, scale=-0.5)
    for e in (0, 1, 2):
        nc.scalar.activation(E[:, :, e], sp[:], AF.Exp, scale=-2.0 * float(ETAS7[e]))

    # fp32 ladders for u^4, v^4, u^16 (bf16 chains compound to >2%/pair)
    u2f = mp.tile([P] + PF, f32)
    nc.scalar.activation(u2f[:], cos[:], AF.Square, bias=0.5, scale=0.5)
    u4b = mp.tile([P] + PF, bf16)
    nc.scalar.activation(u4b[:], u2f[:], AF.Square)
    v2f = mp.tile([P] + PF, f32)
    nc.scalar.activation(v2f[:], cos[:], AF.Square, bias=0.5, scale=-0.5)
    v4b = mp.tile([P] + PF, bf16)
    nc.scalar.activation(v4b[:], v2f[:], AF.Square)
    u8f = mp.tile([P] + PF, f32)
    nc.scalar.activation(u8f[:], u4b[:], AF.Square)
    u16b = mp.tile([P] + PF, bf16)
    nc.scalar.activation(u16b[:], u8f[:], AF.Square)
    for e in (3, 4, 5, 6):
        nc.scalar.activation(E[:, :, e], sp[:], AF.Exp, scale=-2.0 * float(ETAS7[e]))

    def eview_m(e):
        return _view(E[:, 0, e, 0], [[7 * NPAIR, TILES], [1, NPAIR]])

    # ---------------- cells: one positive sum per output column ----------
    COLOFF = {nm: i for i, nm in enumerate(XCLASSES)}
    PRD = {
        nm: mp.tile([P, TREE_N[nm], TILES, NPAIR], bf16, tag=f"PRD{nm}",
                    name=f"PRD{nm}")
        for nm in XCLASSES
    }
    NDMAX = max(TREE_N.values())
    T1 = mp.tile([P, NDMAX, TILES, 248], bf16)
    T2 = mp.tile([P, NDMAX, TILES, 124], bf16)
    T3 = mp.tile([P, NDMAX, TILES, 62], f32)
    T4 = mp.tile([P, NDMAX, TILES, 31], f32)
    NSCRA = 6
    scrA = [mp.tile([P, TILES, NPAIR], bf16, tag=f"scrA{i}", name=f"scrA{i}")
            for i in range(NSCRA)]
    scrAo = mp.tile([P, NPAIR], bf16, tag="scrAo")

    Pt = {}

    def pmake(nm, b0, b1):
        pk = mp.tile([P] + PF, bf16, tag=f"P{nm}", name=f"P{nm}")
        nc.vector.tensor_tensor(out=pk[:], in0=b0[:], in1=b1[:], op=ALU.mult)
        Pt[nm] = pk

    na = [0]

    def emit_act_unit(e, nm, pt):
        peng = nc.gpsimd if (e, nm) in POOL_PRODUCTS else nc.vector
        sA = scrA[na[0] % NSCRA]
        na[0] += 1
        peng.tensor_tensor(out=sA[:], in0=pt[:], in1=eview_m(e), op=ALU.mult)
        col = 50 if nm == "u16" else 8 + 6 * e + COLOFF[nm]
        for t in range(TILES):
            nc.scalar.activation(
                scrAo[:], sA[:, t], AF.Identity,
                accum_out=OUT51[:, t, col : col + 1],
            )

    def emit_tree(nm):
        nd = TREE_N[nm]
        if nd == 0:
            return

        def lvl(src_t, srclen, dst):
            half = srclen // 2
            i0 = _view(src_t[:, 0, 0, 0], [[TILES * srclen, nd], [srclen, TILES], [1, half]])
            i1 = _view(src_t[:, 0, 0, half], [[TILES * srclen, nd], [srclen, TILES], [1, half]])
            o = _view(dst[:, 0, 0, 0], [[TILES * half, nd], [half, TILES], [1, half]])
            nc.vector.tensor_tensor(out=o, in0=i0, in1=i1, op=ALU.add)

        lvl(PRD[nm], NPAIR, T1)
        lvl(T1, 248, T2)
        lvl(T2, 124, T3)
        lvl(T3, 62, T4)
        t4v = _view(T4[:, 0, 0, 0], [[TILES * 31, nd], [31, TILES], [1, 31]])
        mo = _view(OUT51[:, 0, 8 + COLOFF[nm]], [[6, nd], [51, TILES], [0, 1]])
        nc.vector.tensor_reduce(out=mo, in_=t4v, axis=mybir.AxisListType.X, op=ALU.add)

    def emit_class_products(nm):
        for e in range(7):
            if e < TREE_N[nm]:
                nc.vector.tensor_tensor(
                    out=PRD[nm][:, e], in0=Pt[nm][:], in1=eview_m(e), op=ALU.mult
                )
            else:
                emit_act_unit(e, nm, Pt[nm])

    # stage 1: u1/v1 (only need wb/ub/vb) — earliest possible
    pmake("u1", wb, ub)
    pmake("v1", wb, vb)
    emit_class_products("u1")
    emit_class_products("v1")
    emit_tree("u1")
    emit_tree("v1")
    # stage 2: u2/v2 (chained off u1/v1)
    pmake("u2", Pt["u1"], ub)
    pmake("v2", Pt["v1"], vb)
    emit_class_products("u2")
    emit_class_products("v2")
    emit_tree("u2")
    emit_tree("v2")
    # stage 3: u4/v4/u16 (need the fp32 ladders)
    pmake("u4", wb, u4b)
    pmake("v4", wb, v4b)
    P16 = mp.tile([P] + PF, bf16)
    nc.vector.tensor_tensor(out=P16[:], in0=wb[:], in1=u16b[:], op=ALU.mult)
    emit_class_products("u4")
    emit_class_products("v4")
    emit_act_unit(6, "u16", P16)
    emit_tree("u4")
    emit_tree("v4")

    # ---------------- G2 tail (mult + reduce fills late Pool/DVE idle) ----
    G2P = mp.tile([P, TILES, 8, K], f32)
    fc_b = _view(Sm[:, 0, 0, 0], [[7 * K, TILES], [0, 8], [1, K]])
    nc.gpsimd.tensor_tensor(out=G2P[:], in0=E2[:], in1=fc_b, op=ALU.mult)
    out_g2 = _view(OUT51[:, 0, 0], [[51, TILES], [1, 8]])
    nc.vector.tensor_reduce(out=out_g2, in_=G2P[:], axis=mybir.AxisListType.X, op=ALU.add)

    nc.sync.dma_start(out=out_d[:], in_=OUT51[:])


_NC_CACHE = None


def _get_nc():
    global _NC_CACHE
    if _NC_CACHE is None:
        _NC_CACHE = build_nc()
    return _NC_CACHE


def make_inputs(pos, numnei, neighs):
    """Host-side shard prep (data marshalling only, no arithmetic):
    expand pos[neighs] into per-core [P, TILES, K, 3] blocks with sentinel
    rows for invalid neighbor slots and padding atoms."""
    pos = np.asarray(pos, np.float32)
    numnei = np.asarray(numnei, np.int32)
    neighs = np.asarray(neighs, np.int32)
    idx = neighs.reshape(N_ATOMS, K).copy()
    kk = np.arange(K)[None, :]
    invalid = kk >= numnei[:, None]
    gp_full = pos[idx]                      # [N, K, 3] gather (marshalling)
    gp_full[invalid] = FARPOS

    in_maps = []
    for c in range(N_CORES):
        gpd = np.full((P, TILES, K, 3), FARPOS, np.float32)
        ownd = np.zeros((P, TILES, 3), np.float32)
        for t in range(TILES):
            g0 = c * APC + t * P
            n = min(P, APC - t * P)
            if n <= 0:
                continue
            gpd[:n, t] = gp_full[g0 : g0 + n]
            ownd[:n, t] = pos[g0 : g0 + n]
        ctab = np.zeros((P, 16), np.float32)
        for i, v in enumerate([np.pi / 2, -np.pi / 2, 0.5, 0.0, 2e-4]):
            ctab[:, i] = v
        ctab[:, 8:16] = -np.asarray(G2_ETA, np.float32)[None, :]
        in_maps.append({"gpos": gpd, "own": ownd, "ctab": ctab})
    return in_maps


def unshard_output(results):
    out = np.empty((N_ATOMS, 51), np.float32)
    for c in range(N_CORES):
        o = results[c]["out"]            # [P, TILES, 51]
        for t in range(TILES):
            g0 = c * APC + t * P
            n = min(P, APC - t * P)
            if n <= 0:
                continue
            out[g0 : g0 + n] = o[:n, t]
    return out


def run(pos, numnei, neighs, trace=False):
    nc = _get_nc()
    in_maps = make_inputs(pos, numnei, neighs)
    res = run_bass_kernel_spmd(nc, in_maps, list(range(N_CORES)), trace=trace)
    return unshard_output(res.results), res


def kernel(pos, numnei, neighs):
    out, _ = run(pos, numnei, neighs)
    return out
